# revision 16
# baseline (speedup 1.0000x reference)
"""GAT layer kernel for Trainium2, 8 NeuronCores, sharded over the node dim N.

reference computation (per (b, n) node, K=10 neighbors, H=16 heads, d=4):
    hw[k, h, d]  = sum_f x[b,n,k,f] * W[f, h*4+d]
    logit[k, h]  = sum_d hw[k,h,d] * a[d,h]
    attn         = softmax_k(logit)                  (max-subtraction skipped:
                                                      |logit| <~ 6 for randn inputs)
    out[h, d]    = leaky_relu(sum_k attn[k,h] * hw[k,h,d], 0.2)
    output layout [B, 4H, N]

Device strategy per core (rows = B*N_loc*K = 250000 rows of 128 features):
  - load x chunks natural [128 rows, 128 f] (SWDGE DMA with fp32->bf16 cast)
  - PE transpose -> xT [128 f, rows] (bf16, via PSUM + ACT copy to SBUF)
  - per 960-row slice-pair: 4 matmuls pack two 480-row slices onto the 128
    psum partitions (cols 0-63 = slice A, 64-127 = slice B) for both
    hw (lhsT = W) and logits (lhsT = Wa4, `a` folded into W on host,
    replicated x4 so logits align with hw channels)
  - ACT: p = exp(logits); DVE: q = p * hw; grouped reduce over K=10 ->
    numer/denom; reciprocal + multiply + leaky-relu; DMA out [64, nodes]
"""

import numpy as np

try:
    import concourse.bass as bass  # noqa: F401
except ImportError:  # toolchain lives in /opt on these boxes
    import sys

    for p in ("/opt/trn_rl_repo", "/root/.axon_site/_ro/trn_rl_repo"):
        sys.path.insert(0, p)
    import concourse.bass as bass

import concourse.bacc as bacc
import concourse.tile as tile
from concourse import mybir
from concourse.bass import ds  # noqa: F401
from concourse.bass_utils import run_bass_kernel_spmd
from concourse.masks import make_identity

# problem shape (hardcoded per contest rules)
B, N, K, F, H = 4, 50000, 10, 128, 16
C = 4 * H  # 64 output channels
NCORES = 8
NLOC = N // NCORES  # 6250 nodes per core
ROWS = B * NLOC * K  # 250000 rows per core
NODES = B * NLOC  # 25000 nodes per core

CHUNK = 7680  # rows per chunk: 60 transpose tiles, 16 slices, 8 pairs
SLICE = 480  # rows per softmax slice (48 K-groups); one psum bank
TGROUP = 8  # transpose tiles batched per PSUM->SBUF copy (8*128*2B = one bank)

F32 = mybir.dt.float32
BF16 = mybir.dt.bfloat16
QDT = F32  # dtype for exp/attention intermediates (p4, q)
AX = mybir.AxisListType
OP = mybir.AluOpType
AF = mybir.ActivationFunctionType


def _ceil_div(a, b):
    return -(-a // b)


def build_kernel():
    nc = bacc.Bacc("TRN2", target_bir_lowering=False, debug=False)

    x_d = nc.dram_tensor("x", [ROWS, F], F32, kind="ExternalInput")
    wt_d = nc.dram_tensor("wt", [F, 2 * C], BF16, kind="ExternalInput")
    out_d = nc.dram_tensor("out", [C, NODES], F32, kind="ExternalOutput")

    from contextlib import ExitStack

    with tile.TileContext(nc) as tc, ExitStack() as ctx:
        consts = ctx.enter_context(tc.tile_pool(name="consts", bufs=1))
        xnat_p = ctx.enter_context(tc.tile_pool(name="xnat", bufs=2))
        xt_p = ctx.enter_context(tc.tile_pool(name="xt", bufs=2))
        tp_p = ctx.enter_context(tc.tile_pool(name="tp", bufs=2, space="PSUM"))
        hwl_p = ctx.enter_context(tc.tile_pool(name="hwl", bufs=2, space="PSUM"))
        lg_p = ctx.enter_context(tc.tile_pool(name="lg", bufs=2, space="PSUM"))
        p4_p = ctx.enter_context(tc.tile_pool(name="p4", bufs=2))
        q_p = ctx.enter_context(tc.tile_pool(name="q", bufs=2))
        ns_p = ctx.enter_context(tc.tile_pool(name="ns", bufs=2))
        out_p = ctx.enter_context(tc.tile_pool(name="outp", bufs=2))

        wt_sb = consts.tile([F, 2 * C], BF16)
        nc.sync.dma_start(out=wt_sb, in_=wt_d[:, :])
        ident = consts.tile([128, 128], BF16)
        make_identity(nc, ident)

        w_ap = wt_sb[:, 0:C]  # projection weights, cols (h*4+d)
        wa_ap = wt_sb[:, C : 2 * C]  # logit weights replicated x4

        n_chunks = _ceil_div(ROWS, CHUNK)
        for ci in range(n_chunks):
            r0 = ci * CHUNK
            nrows = min(CHUNK, ROWS - r0)
            ntiles = _ceil_div(nrows, 128)
            nfull = nrows // 128

            # ---- load x natural [128 rows, tile, f] with fp32->bf16 cast ----
            xnat = xnat_p.tile([128, CHUNK // 128, F], BF16, tag="xnat")
            if nfull:
                src = x_d[r0 : r0 + nfull * 128, :].rearrange(
                    "(t p) f -> p t f", p=128
                )
                nc.gpsimd.dma_start(out=xnat[:, 0:nfull, :], in_=src)
            if nfull != ntiles:  # partial last tile (end of stream)
                rem = nrows - nfull * 128
                nc.gpsimd.dma_start(
                    out=xnat[0:rem, nfull, :],
                    in_=x_d[r0 + nfull * 128 : r0 + nrows, :],
                )

            # ---- transpose to xT [128 f, rows] via PE + ACT copy ----
            xt = xt_p.tile([F, CHUNK], BF16, tag="xt")
            for g0 in range(0, ntiles, TGROUP):
                gtiles = min(TGROUP, ntiles - g0)
                tp = tp_p.tile([128, TGROUP * 128], BF16, tag="tp")
                width = 0
                for j in range(gtiles):
                    t = g0 + j
                    rt = min(128, nrows - t * 128)
                    nc.tensor.transpose(
                        tp[:, j * 128 : j * 128 + rt],
                        xnat[0:rt, t, :],
                        ident[0:rt, 0:rt],
                    )
                    width = j * 128 + rt
                nc.scalar.copy(
                    out=xt[:, g0 * 128 : g0 * 128 + width], in_=tp[:, 0:width]
                )

            # ---- per slice-pair: matmuls + softmax-weighted combine ----
            nslice = _ceil_div(nrows, SLICE)
            npairs = nslice // 2
            odd = nslice % 2  # trailing unpaired slice (last chunk only)

            p4c = p4_p.tile([128, (CHUNK // SLICE // 2) * SLICE], QDT, tag="p4")
            qc = q_p.tile([128, (CHUNK // SLICE // 2) * SLICE], QDT, tag="q")

            # super-pairs of 2 pairs: hwl spans 2 psum banks so the DVE
            # mult runs once per 1920 rows instead of per 960
            BANK = 512  # fp32 elements per psum bank; MM output can't cross
            for spi in range(0, npairs, 2):
                spn = min(2, npairs - spi)
                hwl = hwl_p.tile([128, 2 * BANK], F32, tag="hwl")
                for pj in range(spn):
                    pi = spi + pj
                    sA = (2 * pi) * SLICE
                    sB = sA + SLICE
                    szA = SLICE
                    szB = min(SLICE, nrows - sB)
                    hslot = pj * BANK
                    lg = lg_p.tile([128, SLICE], F32, tag="lg")
                    nc.tensor.matmul(
                        hwl[0:C, hslot : hslot + szA], w_ap,
                        xt[:, sA : sA + szA], tile_position=(0, 0),
                    )
                    nc.tensor.matmul(
                        hwl[C:128, hslot : hslot + szB], w_ap,
                        xt[:, sB : sB + szB], tile_position=(0, 64),
                    )
                    nc.tensor.matmul(
                        lg[0:C, 0:szA], wa_ap, xt[:, sA : sA + szA],
                        tile_position=(0, 0),
                    )
                    nc.tensor.matmul(
                        lg[C:128, 0:szB], wa_ap, xt[:, sB : sB + szB],
                        tile_position=(0, 64),
                    )
                    # p = exp(logits)  (lanes 0-63 = A, 64-127 = B)
                    nc.scalar.activation(
                        p4c[:, pi * SLICE : pi * SLICE + szA], lg[:, 0:szA],
                        AF.Exp,
                    )
                # q = p * hw for both pairs in one DVE op (hwl slots are
                # bank-aligned at 512; qc/p4c slots are dense at 480)
                qv = qc[:, spi * SLICE : (spi + spn) * SLICE].rearrange(
                    "p (j s) -> p j s", s=SLICE
                )
                pv = p4c[:, spi * SLICE : (spi + spn) * SLICE].rearrange(
                    "p (j s) -> p j s", s=SLICE
                )
                hv = hwl.rearrange("p (j s) -> p j s", s=BANK)[:, 0:spn, 0:SLICE]
                nc.vector.tensor_mul(qv, pv, hv)

            # chunk-level grouped reductions over K for all full pairs
            if npairs:
                gtot = npairs * (SLICE // K)  # groups per lane
                numer = ns_p.tile([128, (CHUNK // SLICE // 2) * (SLICE // K)],
                                  F32, tag="numer")
                denom = ns_p.tile([128, (CHUNK // SLICE // 2) * (SLICE // K)],
                                  F32, tag="denom")
                rinv = ns_p.tile([128, (CHUNK // SLICE // 2) * (SLICE // K)],
                                 F32, tag="rinv")
                outc = out_p.tile([128, (CHUNK // SLICE // 2) * (SLICE // K)],
                                  F32, tag="outc")
                # k-reduction as a pairwise tensor_add tree: tensor_reduce
                # runs at ~0.5 elem/cycle on DVE, tensor_tensor at 1/cycle.
                def ksum_tree(src, dst, tag):
                    v = src[:, 0 : npairs * SLICE].rearrange(
                        "p (pr g k) -> p pr g k", k=K, g=SLICE // K
                    )
                    t5 = ns_p.tile(
                        [128, (CHUNK // SLICE // 2) * (SLICE // K) * 5],
                        F32, tag=tag + "5")
                    t5v = t5[:, 0 : gtot * 5].rearrange(
                        "p (pr g k) -> p pr g k", k=5, g=SLICE // K)
                    nc.vector.tensor_add(t5v, v[:, :, :, 0:5], v[:, :, :, 5:10])
                    t2 = ns_p.tile(
                        [128, (CHUNK // SLICE // 2) * (SLICE // K) * 2],
                        F32, tag=tag + "2")
                    t2v = t2[:, 0 : gtot * 2].rearrange(
                        "p (pr g k) -> p pr g k", k=2, g=SLICE // K)
                    nc.vector.tensor_add(
                        t2v, t5v[:, :, :, 0:2], t5v[:, :, :, 2:4])
                    dv = dst[:, 0:gtot].rearrange(
                        "p (pr g) -> p pr g", g=SLICE // K)
                    nc.vector.tensor_add(
                        dv, t2v[:, :, :, 0], t2v[:, :, :, 1])
                    nc.vector.tensor_add(dv, dv, t5v[:, :, :, 4])

                ksum_tree(qc, numer, "qt")
                ksum_tree(p4c, denom, "pt")
                nc.vector.reciprocal_approx_fast(rinv[:, 0:gtot], denom[:, 0:gtot])
                nc.vector.tensor_mul(
                    outc[:, 0:gtot], numer[:, 0:gtot], rinv[:, 0:gtot]
                )
                # leaky relu: out = max(0.2*x, x)
                nc.vector.scalar_tensor_tensor(
                    out=outc[:, 0:gtot], in0=outc[:, 0:gtot], scalar=0.2,
                    in1=outc[:, 0:gtot], op0=OP.mult, op1=OP.max,
                )

                nb = r0 // K
                gp = SLICE // K  # 48 nodes per slice
                dstA = out_d[:, nb : nb + npairs * 2 * gp].rearrange(
                    "c (pr g2) -> c pr g2", g2=2 * gp
                )
                nc.sync.dma_start(
                    out=dstA[:, :, 0:gp],
                    in_=outc[0:C, 0:gtot].rearrange("c (pr g) -> c pr g", g=gp),
                )
                nc.sync.dma_start(
                    out=dstA[:, :, gp : 2 * gp],
                    in_=outc[C:128, 0:gtot].rearrange("c (pr g) -> c pr g", g=gp),
                )

            if odd:
                sA = (nslice - 1) * SLICE
                szA = nrows - sA
                go = szA // K
                hwl = hwl_p.tile([128, SLICE], F32, tag="hwl")
                lg = lg_p.tile([128, SLICE], F32, tag="lg")
                nc.tensor.matmul(
                    hwl[0:C, 0:szA], w_ap, xt[:, sA : sA + szA],
                    tile_position=(0, 0),
                )
                nc.tensor.matmul(
                    lg[0:C, 0:szA], wa_ap, xt[:, sA : sA + szA],
                    tile_position=(0, 0),
                )
                p4o = p4_p.tile([C, SLICE], QDT, tag="p4o")
                qo = q_p.tile([C, SLICE], QDT, tag="qo")
                nc.scalar.activation(p4o[:, 0:szA], lg[0:C, 0:szA], AF.Exp)
                nc.vector.tensor_mul(qo[:, 0:szA], p4o[:, 0:szA], hwl[0:C, 0:szA])
                nso = ns_p.tile([C, 4 * (SLICE // K)], F32, tag="nso")
                nc.vector.tensor_reduce(
                    nso[:, 0:go],
                    qo[:, 0:szA].rearrange("p (g k) -> p g k", k=K),
                    axis=AX.X, op=OP.add,
                )
                nc.vector.tensor_reduce(
                    nso[:, go : 2 * go],
                    p4o[:, 0:szA].rearrange("p (g k) -> p g k", k=K),
                    axis=AX.X, op=OP.add,
                )
                nc.vector.reciprocal(nso[:, 2 * go : 3 * go], nso[:, go : 2 * go])
                nc.vector.tensor_mul(
                    nso[:, 3 * go : 4 * go], nso[:, 0:go],
                    nso[:, 2 * go : 3 * go],
                )
                nc.vector.scalar_tensor_tensor(
                    out=nso[:, 3 * go : 4 * go], in0=nso[:, 3 * go : 4 * go],
                    scalar=0.2, in1=nso[:, 3 * go : 4 * go],
                    op0=OP.mult, op1=OP.max,
                )
                nb = (r0 + sA) // K
                nc.sync.dma_start(
                    out=out_d[:, nb : nb + go], in_=nso[:, 3 * go : 4 * go]
                )

    nc.compile()
    return nc


_NC = None


def _get_nc():
    global _NC
    if _NC is None:
        _NC = build_kernel()
    return _NC


def kernel(x, W, a):
    import ml_dtypes

    x = np.asarray(x, dtype=np.float32)
    W = np.asarray(W, dtype=np.float32)
    a = np.asarray(a, dtype=np.float32)

    # fold `a` into W: Wa[f, h] = sum_d W[f, h*4+d] * a[d, h]; replicate x4
    Wr = W.reshape(F, H, 4)
    Wa = np.einsum("fhd,dh->fh", Wr.astype(np.float64), a.astype(np.float64))
    Wa4 = np.repeat(Wa, 4, axis=1)  # [F, 64], col h*4+d -> Wa[:, h]
    wt = np.concatenate([W, Wa4.astype(np.float32)], axis=1)
    wt_bf16 = wt.astype(ml_dtypes.bfloat16)

    nc = _get_nc()
    in_maps = []
    for i in range(NCORES):
        xi = np.ascontiguousarray(x[:, i * NLOC : (i + 1) * NLOC])
        in_maps.append({"x": xi.reshape(ROWS, F), "wt": wt_bf16})

    res = run_bass_kernel_spmd(nc, in_maps, core_ids=list(range(NCORES)))
    outs = []
    for i in range(NCORES):
        o = res.results[i]["out"]  # [C, NODES] with node index = b*NLOC + n
        outs.append(o.reshape(C, B, NLOC).transpose(1, 0, 2))  # [B, C, NLOC]
    return np.concatenate(outs, axis=2)  # [B, C, N]


# revision 20
# speedup vs baseline: 1.0638x; 1.0638x over previous
"""GAT layer kernel for Trainium2, 8 NeuronCores, sharded over the node dim N.

reference computation (per (b, n) node, K=10 neighbors, H=16 heads, d=4):
    hw[k, h, d]  = sum_f x[b,n,k,f] * W[f, h*4+d]
    logit[k, h]  = sum_d hw[k,h,d] * a[d,h]
    attn         = softmax_k(logit)                  (max-subtraction skipped:
                                                      |logit| <~ 6 for randn inputs)
    out[h, d]    = leaky_relu(sum_k attn[k,h] * hw[k,h,d], 0.2)
    output layout [B, 4H, N]

Device strategy per core (rows = B*N_loc*K = 250000 rows of 128 features):
  - load x chunks natural [128 rows, 128 f] (SWDGE DMA with fp32->bf16 cast)
  - PE transpose -> xT [128 f, rows] (bf16, via PSUM + ACT copy to SBUF)
  - per 960-row slice-pair: 4 matmuls pack two 480-row slices onto the 128
    psum partitions (cols 0-63 = slice A, 64-127 = slice B) for both
    hw (lhsT = W) and logits (lhsT = Wa4, `a` folded into W on host,
    replicated x4 so logits align with hw channels)
  - ACT: p = exp(logits); DVE: q = p * hw; grouped reduce over K=10 ->
    numer/denom; reciprocal + multiply + leaky-relu; DMA out [64, nodes]
"""

import numpy as np

try:
    import concourse.bass as bass  # noqa: F401
except ImportError:  # toolchain lives in /opt on these boxes
    import sys

    for p in ("/opt/trn_rl_repo", "/root/.axon_site/_ro/trn_rl_repo"):
        sys.path.insert(0, p)
    import concourse.bass as bass

import concourse.bacc as bacc
import concourse.tile as tile
from concourse import mybir
from concourse.bass import ds  # noqa: F401
from concourse.bass_utils import run_bass_kernel_spmd
from concourse.masks import make_identity

# problem shape (hardcoded per contest rules)
B, N, K, F, H = 4, 50000, 10, 128, 16
C = 4 * H  # 64 output channels
NCORES = 8
NLOC = N // NCORES  # 6250 nodes per core
ROWS = B * NLOC * K  # 250000 rows per core
NODES = B * NLOC  # 25000 nodes per core

CHUNK = 7680  # rows per chunk: 60 transpose tiles, 16 slices, 8 pairs
SLICE = 480  # rows per softmax slice (48 K-groups); one psum bank
TGROUP = 8  # transpose tiles batched per PSUM->SBUF copy (8*128*2B = one bank)

F32 = mybir.dt.float32
BF16 = mybir.dt.bfloat16
QDT = F32  # dtype for exp/attention intermediates (p4, q)
AX = mybir.AxisListType
OP = mybir.AluOpType
AF = mybir.ActivationFunctionType


def _ceil_div(a, b):
    return -(-a // b)


def build_kernel():
    nc = bacc.Bacc("TRN2", target_bir_lowering=False, debug=False)

    x_d = nc.dram_tensor("x", [ROWS, F], F32, kind="ExternalInput")
    wt_d = nc.dram_tensor("wt", [F, 2 * C], BF16, kind="ExternalInput")
    out_d = nc.dram_tensor("out", [C, NODES], F32, kind="ExternalOutput")

    from contextlib import ExitStack

    with tile.TileContext(nc) as tc, ExitStack() as ctx:
        consts = ctx.enter_context(tc.tile_pool(name="consts", bufs=1))
        xnat_p = ctx.enter_context(tc.tile_pool(name="xnat", bufs=3))
        xt_p = ctx.enter_context(tc.tile_pool(name="xt", bufs=2))
        tp_p = ctx.enter_context(tc.tile_pool(name="tp", bufs=2, space="PSUM"))
        hwl_p = ctx.enter_context(tc.tile_pool(name="hwl", bufs=2, space="PSUM"))
        lg_p = ctx.enter_context(tc.tile_pool(name="lg", bufs=2, space="PSUM"))
        p4_p = ctx.enter_context(tc.tile_pool(name="p4", bufs=2))
        q_p = ctx.enter_context(tc.tile_pool(name="q", bufs=2))
        ns_p = ctx.enter_context(tc.tile_pool(name="ns", bufs=2))
        out_p = ctx.enter_context(tc.tile_pool(name="outp", bufs=2))

        wt_sb = consts.tile([F, 2 * C], BF16)
        nc.sync.dma_start(out=wt_sb, in_=wt_d[:, :])
        ident = consts.tile([128, 128], BF16)
        make_identity(nc, ident)

        w_ap = wt_sb[:, 0:C]  # projection weights, cols (h*4+d)
        wa_ap = wt_sb[:, C : 2 * C]  # logit weights replicated x4

        n_chunks = _ceil_div(ROWS, CHUNK)
        for ci in range(n_chunks):
            r0 = ci * CHUNK
            nrows = min(CHUNK, ROWS - r0)
            ntiles = _ceil_div(nrows, 128)
            nfull = nrows // 128

            # ---- load x natural [128 rows, tile, f] with fp32->bf16 cast ----
            xnat = xnat_p.tile([128, CHUNK // 128, F], BF16, tag="xnat")
            if nfull:
                src = x_d[r0 : r0 + nfull * 128, :].rearrange(
                    "(t p) f -> p t f", p=128
                )
                nc.gpsimd.dma_start(out=xnat[:, 0:nfull, :], in_=src)
            if nfull != ntiles:  # partial last tile (end of stream)
                rem = nrows - nfull * 128
                nc.gpsimd.dma_start(
                    out=xnat[0:rem, nfull, :],
                    in_=x_d[r0 + nfull * 128 : r0 + nrows, :],
                )

            # ---- transpose to xT [128 f, rows] via PE + ACT copy ----
            xt = xt_p.tile([F, CHUNK], BF16, tag="xt")
            for g0 in range(0, ntiles, TGROUP):
                gtiles = min(TGROUP, ntiles - g0)
                tp = tp_p.tile([128, TGROUP * 128], BF16, tag="tp")
                width = 0
                for j in range(gtiles):
                    t = g0 + j
                    rt = min(128, nrows - t * 128)
                    nc.tensor.transpose(
                        tp[:, j * 128 : j * 128 + rt],
                        xnat[0:rt, t, :],
                        ident[0:rt, 0:rt],
                    )
                    width = j * 128 + rt
                nc.scalar.copy(
                    out=xt[:, g0 * 128 : g0 * 128 + width], in_=tp[:, 0:width]
                )

            # ---- per slice-pair: matmuls + softmax-weighted combine ----
            nslice = _ceil_div(nrows, SLICE)
            npairs = nslice // 2
            odd = nslice % 2  # trailing unpaired slice (last chunk only)

            p4c = p4_p.tile([128, (CHUNK // SLICE // 2) * SLICE], QDT, tag="p4")
            qc = q_p.tile([128, (CHUNK // SLICE // 2) * SLICE], QDT, tag="q")

            # super-pairs of 2 pairs: hwl spans 2 psum banks so the DVE
            # mult runs once per 1920 rows instead of per 960
            BANK = 512  # fp32 elements per psum bank; MM output can't cross
            for spi in range(0, npairs, 2):
                spn = min(2, npairs - spi)
                hwl = hwl_p.tile([128, 2 * BANK], F32, tag="hwl")
                for pj in range(spn):
                    pi = spi + pj
                    sA = (2 * pi) * SLICE
                    sB = sA + SLICE
                    szA = SLICE
                    szB = min(SLICE, nrows - sB)
                    hslot = pj * BANK
                    lg = lg_p.tile([128, SLICE], F32, tag="lg")
                    nc.tensor.matmul(
                        hwl[0:C, hslot : hslot + szA], w_ap,
                        xt[:, sA : sA + szA], tile_position=(0, 0),
                    )
                    nc.tensor.matmul(
                        hwl[C:128, hslot : hslot + szB], w_ap,
                        xt[:, sB : sB + szB], tile_position=(0, 64),
                    )
                    nc.tensor.matmul(
                        lg[0:C, 0:szA], wa_ap, xt[:, sA : sA + szA],
                        tile_position=(0, 0),
                    )
                    nc.tensor.matmul(
                        lg[C:128, 0:szB], wa_ap, xt[:, sB : sB + szB],
                        tile_position=(0, 64),
                    )
                    # p = exp(logits)  (lanes 0-63 = A, 64-127 = B)
                    nc.scalar.activation(
                        p4c[:, pi * SLICE : pi * SLICE + szA], lg[:, 0:szA],
                        AF.Exp,
                    )
                # q = p * hw for both pairs in one DVE op (hwl slots are
                # bank-aligned at 512; qc/p4c slots are dense at 480)
                qv = qc[:, spi * SLICE : (spi + spn) * SLICE].rearrange(
                    "p (j s) -> p j s", s=SLICE
                )
                pv = p4c[:, spi * SLICE : (spi + spn) * SLICE].rearrange(
                    "p (j s) -> p j s", s=SLICE
                )
                hv = hwl.rearrange("p (j s) -> p j s", s=BANK)[:, 0:spn, 0:SLICE]
                nc.vector.tensor_mul(qv, pv, hv)

            # chunk-level grouped reductions over K for all full pairs
            if npairs:
                gtot = npairs * (SLICE // K)  # groups per lane
                numer = ns_p.tile([128, (CHUNK // SLICE // 2) * (SLICE // K)],
                                  F32, tag="numer")
                denom = ns_p.tile([128, (CHUNK // SLICE // 2) * (SLICE // K)],
                                  F32, tag="denom")
                rinv = ns_p.tile([128, (CHUNK // SLICE // 2) * (SLICE // K)],
                                 F32, tag="rinv")
                outc = out_p.tile([128, (CHUNK // SLICE // 2) * (SLICE // K)],
                                  F32, tag="outc")
                q_v = qc[:, 0 : npairs * SLICE].rearrange(
                    "p (pr g k) -> p pr g k", k=K, g=SLICE // K
                )
                p_v = p4c[:, 0 : npairs * SLICE].rearrange(
                    "p (pr g k) -> p pr g k", k=K, g=SLICE // K
                )
                nc.vector.tensor_reduce(
                    numer[:, 0:gtot].rearrange("p (pr g) -> p pr g", g=SLICE // K),
                    q_v, axis=AX.X, op=OP.add,
                )
                nc.vector.tensor_reduce(
                    denom[:, 0:gtot].rearrange("p (pr g) -> p pr g", g=SLICE // K),
                    p_v, axis=AX.X, op=OP.add,
                )
                nc.vector.reciprocal_approx_fast(rinv[:, 0:gtot], denom[:, 0:gtot])
                nc.vector.tensor_mul(
                    outc[:, 0:gtot], numer[:, 0:gtot], rinv[:, 0:gtot]
                )
                # leaky relu: out = max(0.2*x, x)
                nc.vector.scalar_tensor_tensor(
                    out=outc[:, 0:gtot], in0=outc[:, 0:gtot], scalar=0.2,
                    in1=outc[:, 0:gtot], op0=OP.mult, op1=OP.max,
                )

                nb = r0 // K
                gp = SLICE // K  # 48 nodes per slice
                dstA = out_d[:, nb : nb + npairs * 2 * gp].rearrange(
                    "c (pr g2) -> c pr g2", g2=2 * gp
                )
                nc.sync.dma_start(
                    out=dstA[:, :, 0:gp],
                    in_=outc[0:C, 0:gtot].rearrange("c (pr g) -> c pr g", g=gp),
                )
                nc.sync.dma_start(
                    out=dstA[:, :, gp : 2 * gp],
                    in_=outc[C:128, 0:gtot].rearrange("c (pr g) -> c pr g", g=gp),
                )

            if odd:
                sA = (nslice - 1) * SLICE
                szA = nrows - sA
                go = szA // K
                hwl = hwl_p.tile([128, SLICE], F32, tag="hwl")
                lg = lg_p.tile([128, SLICE], F32, tag="lg")
                nc.tensor.matmul(
                    hwl[0:C, 0:szA], w_ap, xt[:, sA : sA + szA],
                    tile_position=(0, 0),
                )
                nc.tensor.matmul(
                    lg[0:C, 0:szA], wa_ap, xt[:, sA : sA + szA],
                    tile_position=(0, 0),
                )
                p4o = p4_p.tile([C, SLICE], QDT, tag="p4o")
                qo = q_p.tile([C, SLICE], QDT, tag="qo")
                nc.scalar.activation(p4o[:, 0:szA], lg[0:C, 0:szA], AF.Exp)
                nc.vector.tensor_mul(qo[:, 0:szA], p4o[:, 0:szA], hwl[0:C, 0:szA])
                nso = ns_p.tile([C, 4 * (SLICE // K)], F32, tag="nso")
                nc.vector.tensor_reduce(
                    nso[:, 0:go],
                    qo[:, 0:szA].rearrange("p (g k) -> p g k", k=K),
                    axis=AX.X, op=OP.add,
                )
                nc.vector.tensor_reduce(
                    nso[:, go : 2 * go],
                    p4o[:, 0:szA].rearrange("p (g k) -> p g k", k=K),
                    axis=AX.X, op=OP.add,
                )
                nc.vector.reciprocal(nso[:, 2 * go : 3 * go], nso[:, go : 2 * go])
                nc.vector.tensor_mul(
                    nso[:, 3 * go : 4 * go], nso[:, 0:go],
                    nso[:, 2 * go : 3 * go],
                )
                nc.vector.scalar_tensor_tensor(
                    out=nso[:, 3 * go : 4 * go], in0=nso[:, 3 * go : 4 * go],
                    scalar=0.2, in1=nso[:, 3 * go : 4 * go],
                    op0=OP.mult, op1=OP.max,
                )
                nb = (r0 + sA) // K
                nc.sync.dma_start(
                    out=out_d[:, nb : nb + go], in_=nso[:, 3 * go : 4 * go]
                )

    nc.compile()
    return nc


_NC = None


def _get_nc():
    global _NC
    if _NC is None:
        _NC = build_kernel()
    return _NC


def kernel(x, W, a):
    import ml_dtypes

    x = np.asarray(x, dtype=np.float32)
    W = np.asarray(W, dtype=np.float32)
    a = np.asarray(a, dtype=np.float32)

    # fold `a` into W: Wa[f, h] = sum_d W[f, h*4+d] * a[d, h]; replicate x4
    Wr = W.reshape(F, H, 4)
    Wa = np.einsum("fhd,dh->fh", Wr.astype(np.float64), a.astype(np.float64))
    Wa4 = np.repeat(Wa, 4, axis=1)  # [F, 64], col h*4+d -> Wa[:, h]
    wt = np.concatenate([W, Wa4.astype(np.float32)], axis=1)
    wt_bf16 = wt.astype(ml_dtypes.bfloat16)

    nc = _get_nc()
    in_maps = []
    for i in range(NCORES):
        xi = np.ascontiguousarray(x[:, i * NLOC : (i + 1) * NLOC])
        in_maps.append({"x": xi.reshape(ROWS, F), "wt": wt_bf16})

    res = run_bass_kernel_spmd(nc, in_maps, core_ids=list(range(NCORES)))
    outs = []
    for i in range(NCORES):
        o = res.results[i]["out"]  # [C, NODES] with node index = b*NLOC + n
        outs.append(o.reshape(C, B, NLOC).transpose(1, 0, 2))  # [B, C, NLOC]
    return np.concatenate(outs, axis=2)  # [B, C, N]


# revision 22
# speedup vs baseline: 1.2046x; 1.1323x over previous
"""GAT layer kernel for Trainium2, 8 NeuronCores, sharded over the node dim N.

reference computation (per (b, n) node, K=10 neighbors, H=16 heads, d=4):
    hw[k, h, d]  = sum_f x[b,n,k,f] * W[f, h*4+d]
    logit[k, h]  = sum_d hw[k,h,d] * a[d,h]
    attn         = softmax_k(logit)                  (max-subtraction skipped:
                                                      |logit| <~ 6 for randn inputs)
    out[h, d]    = leaky_relu(sum_k attn[k,h] * hw[k,h,d], 0.2)
    output layout [B, 4H, N]

Device strategy per core (rows = B*N_loc*K = 250000 rows of 128 features):
  - load x chunks natural [128 rows, 128 f] (SWDGE DMA with fp32->bf16 cast)
  - PE transpose -> xT [128 f, rows] (bf16, via PSUM + ACT copy to SBUF)
  - per 960-row slice-pair: 4 matmuls pack two 480-row slices onto the 128
    psum partitions (cols 0-63 = slice A, 64-127 = slice B) for both
    hw (lhsT = W) and logits (lhsT = Wa4, `a` folded into W on host,
    replicated x4 so logits align with hw channels)
  - ACT: p = exp(logits); DVE: q = p * hw; grouped reduce over K=10 ->
    numer/denom; reciprocal + multiply + leaky-relu; DMA out [64, nodes]
"""

import numpy as np

try:
    import concourse.bass as bass  # noqa: F401
except ImportError:  # toolchain lives in /opt on these boxes
    import sys

    for p in ("/opt/trn_rl_repo", "/root/.axon_site/_ro/trn_rl_repo"):
        sys.path.insert(0, p)
    import concourse.bass as bass

import concourse.bacc as bacc
import concourse.tile as tile
from concourse import mybir
from concourse.bass import ds  # noqa: F401
from concourse.bass_utils import run_bass_kernel_spmd
from concourse.masks import make_identity

# problem shape (hardcoded per contest rules)
B, N, K, F, H = 4, 50000, 10, 128, 16
C = 4 * H  # 64 output channels
NCORES = 8
NLOC = N // NCORES  # 6250 nodes per core
ROWS = B * NLOC * K  # 250000 rows per core
NODES = B * NLOC  # 25000 nodes per core

CHUNK = 7680  # rows per chunk: 60 transpose tiles, 16 slices, 8 pairs
SLICE = 480  # rows per softmax slice (48 K-groups); one psum bank
TGROUP = 8  # transpose tiles batched per PSUM->SBUF copy (8*128*2B = one bank)

F32 = mybir.dt.float32
BF16 = mybir.dt.bfloat16
QDT = BF16  # dtype for exp/attention intermediates (p4, q)
AX = mybir.AxisListType
OP = mybir.AluOpType
AF = mybir.ActivationFunctionType


def _ceil_div(a, b):
    return -(-a // b)


def build_kernel():
    nc = bacc.Bacc("TRN2", target_bir_lowering=False, debug=False)

    x_d = nc.dram_tensor("x", [ROWS, F], F32, kind="ExternalInput")
    wt_d = nc.dram_tensor("wt", [F, 2 * C], BF16, kind="ExternalInput")
    out_d = nc.dram_tensor("out", [C, NODES], F32, kind="ExternalOutput")

    from contextlib import ExitStack

    with tile.TileContext(nc) as tc, ExitStack() as ctx:
        consts = ctx.enter_context(tc.tile_pool(name="consts", bufs=1))
        xnat_p = ctx.enter_context(tc.tile_pool(name="xnat", bufs=3))
        xt_p = ctx.enter_context(tc.tile_pool(name="xt", bufs=3))
        tp_p = ctx.enter_context(tc.tile_pool(name="tp", bufs=2, space="PSUM"))
        hwl_p = ctx.enter_context(tc.tile_pool(name="hwl", bufs=2, space="PSUM"))
        lg_p = ctx.enter_context(tc.tile_pool(name="lg", bufs=2, space="PSUM"))
        p4_p = ctx.enter_context(tc.tile_pool(name="p4", bufs=3))
        q_p = ctx.enter_context(tc.tile_pool(name="q", bufs=3))
        ns_p = ctx.enter_context(tc.tile_pool(name="ns", bufs=3))
        out_p = ctx.enter_context(tc.tile_pool(name="outp", bufs=3))

        wt_sb = consts.tile([F, 2 * C], BF16)
        nc.sync.dma_start(out=wt_sb, in_=wt_d[:, :])
        ident = consts.tile([128, 128], BF16)
        make_identity(nc, ident)

        w_ap = wt_sb[:, 0:C]  # projection weights, cols (h*4+d)
        wa_ap = wt_sb[:, C : 2 * C]  # logit weights replicated x4

        n_chunks = _ceil_div(ROWS, CHUNK)
        for ci in range(n_chunks):
            r0 = ci * CHUNK
            nrows = min(CHUNK, ROWS - r0)
            ntiles = _ceil_div(nrows, 128)
            nfull = nrows // 128

            # ---- load x natural [128 rows, tile, f] with fp32->bf16 cast ----
            xnat = xnat_p.tile([128, CHUNK // 128, F], BF16, tag="xnat")
            if nfull:
                src = x_d[r0 : r0 + nfull * 128, :].rearrange(
                    "(t p) f -> p t f", p=128
                )
                nc.gpsimd.dma_start(out=xnat[:, 0:nfull, :], in_=src)
            if nfull != ntiles:  # partial last tile (end of stream)
                rem = nrows - nfull * 128
                nc.gpsimd.dma_start(
                    out=xnat[0:rem, nfull, :],
                    in_=x_d[r0 + nfull * 128 : r0 + nrows, :],
                )

            # ---- transpose to xT [128 f, rows] via PE + ACT copy ----
            xt = xt_p.tile([F, CHUNK], BF16, tag="xt")
            for g0 in range(0, ntiles, TGROUP):
                gtiles = min(TGROUP, ntiles - g0)
                tp = tp_p.tile([128, TGROUP * 128], BF16, tag="tp")
                width = 0
                for j in range(gtiles):
                    t = g0 + j
                    rt = min(128, nrows - t * 128)
                    nc.tensor.transpose(
                        tp[:, j * 128 : j * 128 + rt],
                        xnat[0:rt, t, :],
                        ident[0:rt, 0:rt],
                    )
                    width = j * 128 + rt
                nc.scalar.copy(
                    out=xt[:, g0 * 128 : g0 * 128 + width], in_=tp[:, 0:width]
                )

            # ---- per slice-pair: matmuls + softmax-weighted combine ----
            nslice = _ceil_div(nrows, SLICE)
            npairs = nslice // 2
            odd = nslice % 2  # trailing unpaired slice (last chunk only)

            p4c = p4_p.tile([128, (CHUNK // SLICE // 2) * SLICE], QDT, tag="p4")
            qc = q_p.tile([128, (CHUNK // SLICE // 2) * SLICE], QDT, tag="q")

            # super-pairs of 2 pairs: hwl spans 2 psum banks so the DVE
            # mult runs once per 1920 rows instead of per 960
            BANK = 512  # fp32 elements per psum bank; MM output can't cross
            for spi in range(0, npairs, 2):
                spn = min(2, npairs - spi)
                hwl = hwl_p.tile([128, 2 * BANK], F32, tag="hwl")
                for pj in range(spn):
                    pi = spi + pj
                    sA = (2 * pi) * SLICE
                    sB = sA + SLICE
                    szA = SLICE
                    szB = min(SLICE, nrows - sB)
                    hslot = pj * BANK
                    lg = lg_p.tile([128, SLICE], F32, tag="lg")
                    nc.tensor.matmul(
                        hwl[0:C, hslot : hslot + szA], w_ap,
                        xt[:, sA : sA + szA], tile_position=(0, 0),
                    )
                    nc.tensor.matmul(
                        hwl[C:128, hslot : hslot + szB], w_ap,
                        xt[:, sB : sB + szB], tile_position=(0, 64),
                    )
                    nc.tensor.matmul(
                        lg[0:C, 0:szA], wa_ap, xt[:, sA : sA + szA],
                        tile_position=(0, 0),
                    )
                    nc.tensor.matmul(
                        lg[C:128, 0:szB], wa_ap, xt[:, sB : sB + szB],
                        tile_position=(0, 64),
                    )
                    # p = exp(logits)  (lanes 0-63 = A, 64-127 = B)
                    nc.scalar.activation(
                        p4c[:, pi * SLICE : pi * SLICE + szA], lg[:, 0:szA],
                        AF.Exp,
                    )
                # q = p * hw for both pairs in one DVE op (hwl slots are
                # bank-aligned at 512; qc/p4c slots are dense at 480)
                qv = qc[:, spi * SLICE : (spi + spn) * SLICE].rearrange(
                    "p (j s) -> p j s", s=SLICE
                )
                pv = p4c[:, spi * SLICE : (spi + spn) * SLICE].rearrange(
                    "p (j s) -> p j s", s=SLICE
                )
                hv = hwl.rearrange("p (j s) -> p j s", s=BANK)[:, 0:spn, 0:SLICE]
                nc.vector.tensor_mul(qv, pv, hv)

            # chunk-level grouped reductions over K for all full pairs
            if npairs:
                gtot = npairs * (SLICE // K)  # groups per lane
                numer = ns_p.tile([128, (CHUNK // SLICE // 2) * (SLICE // K)],
                                  F32, tag="numer")
                denom = ns_p.tile([128, (CHUNK // SLICE // 2) * (SLICE // K)],
                                  F32, tag="denom")
                rinv = ns_p.tile([128, (CHUNK // SLICE // 2) * (SLICE // K)],
                                 F32, tag="rinv")
                outc = out_p.tile([128, (CHUNK // SLICE // 2) * (SLICE // K)],
                                  F32, tag="outc")
                q_v = qc[:, 0 : npairs * SLICE].rearrange(
                    "p (pr g k) -> p pr g k", k=K, g=SLICE // K
                )
                p_v = p4c[:, 0 : npairs * SLICE].rearrange(
                    "p (pr g k) -> p pr g k", k=K, g=SLICE // K
                )
                nc.vector.tensor_reduce(
                    numer[:, 0:gtot].rearrange("p (pr g) -> p pr g", g=SLICE // K),
                    q_v, axis=AX.X, op=OP.add,
                )
                nc.vector.tensor_reduce(
                    denom[:, 0:gtot].rearrange("p (pr g) -> p pr g", g=SLICE // K),
                    p_v, axis=AX.X, op=OP.add,
                )
                nc.vector.reciprocal_approx_fast(rinv[:, 0:gtot], denom[:, 0:gtot])
                nc.vector.tensor_mul(
                    outc[:, 0:gtot], numer[:, 0:gtot], rinv[:, 0:gtot]
                )
                # leaky relu: out = max(0.2*x, x)
                nc.vector.scalar_tensor_tensor(
                    out=outc[:, 0:gtot], in0=outc[:, 0:gtot], scalar=0.2,
                    in1=outc[:, 0:gtot], op0=OP.mult, op1=OP.max,
                )

                nb = r0 // K
                gp = SLICE // K  # 48 nodes per slice
                dstA = out_d[:, nb : nb + npairs * 2 * gp].rearrange(
                    "c (pr g2) -> c pr g2", g2=2 * gp
                )
                nc.sync.dma_start(
                    out=dstA[:, :, 0:gp],
                    in_=outc[0:C, 0:gtot].rearrange("c (pr g) -> c pr g", g=gp),
                )
                nc.sync.dma_start(
                    out=dstA[:, :, gp : 2 * gp],
                    in_=outc[C:128, 0:gtot].rearrange("c (pr g) -> c pr g", g=gp),
                )

            if odd:
                sA = (nslice - 1) * SLICE
                szA = nrows - sA
                go = szA // K
                hwl = hwl_p.tile([128, SLICE], F32, tag="hwl")
                lg = lg_p.tile([128, SLICE], F32, tag="lg")
                nc.tensor.matmul(
                    hwl[0:C, 0:szA], w_ap, xt[:, sA : sA + szA],
                    tile_position=(0, 0),
                )
                nc.tensor.matmul(
                    lg[0:C, 0:szA], wa_ap, xt[:, sA : sA + szA],
                    tile_position=(0, 0),
                )
                p4o = p4_p.tile([C, SLICE], QDT, tag="p4o")
                qo = q_p.tile([C, SLICE], QDT, tag="qo")
                nc.scalar.activation(p4o[:, 0:szA], lg[0:C, 0:szA], AF.Exp)
                nc.vector.tensor_mul(qo[:, 0:szA], p4o[:, 0:szA], hwl[0:C, 0:szA])
                nso = ns_p.tile([C, 4 * (SLICE // K)], F32, tag="nso")
                nc.vector.tensor_reduce(
                    nso[:, 0:go],
                    qo[:, 0:szA].rearrange("p (g k) -> p g k", k=K),
                    axis=AX.X, op=OP.add,
                )
                nc.vector.tensor_reduce(
                    nso[:, go : 2 * go],
                    p4o[:, 0:szA].rearrange("p (g k) -> p g k", k=K),
                    axis=AX.X, op=OP.add,
                )
                nc.vector.reciprocal(nso[:, 2 * go : 3 * go], nso[:, go : 2 * go])
                nc.vector.tensor_mul(
                    nso[:, 3 * go : 4 * go], nso[:, 0:go],
                    nso[:, 2 * go : 3 * go],
                )
                nc.vector.scalar_tensor_tensor(
                    out=nso[:, 3 * go : 4 * go], in0=nso[:, 3 * go : 4 * go],
                    scalar=0.2, in1=nso[:, 3 * go : 4 * go],
                    op0=OP.mult, op1=OP.max,
                )
                nb = (r0 + sA) // K
                nc.sync.dma_start(
                    out=out_d[:, nb : nb + go], in_=nso[:, 3 * go : 4 * go]
                )

    nc.compile()
    return nc


_NC = None


def _get_nc():
    global _NC
    if _NC is None:
        _NC = build_kernel()
    return _NC


def kernel(x, W, a):
    import ml_dtypes

    x = np.asarray(x, dtype=np.float32)
    W = np.asarray(W, dtype=np.float32)
    a = np.asarray(a, dtype=np.float32)

    # fold `a` into W: Wa[f, h] = sum_d W[f, h*4+d] * a[d, h]; replicate x4
    Wr = W.reshape(F, H, 4)
    Wa = np.einsum("fhd,dh->fh", Wr.astype(np.float64), a.astype(np.float64))
    Wa4 = np.repeat(Wa, 4, axis=1)  # [F, 64], col h*4+d -> Wa[:, h]
    wt = np.concatenate([W, Wa4.astype(np.float32)], axis=1)
    wt_bf16 = wt.astype(ml_dtypes.bfloat16)

    nc = _get_nc()
    in_maps = []
    for i in range(NCORES):
        xi = np.ascontiguousarray(x[:, i * NLOC : (i + 1) * NLOC])
        in_maps.append({"x": xi.reshape(ROWS, F), "wt": wt_bf16})

    res = run_bass_kernel_spmd(nc, in_maps, core_ids=list(range(NCORES)))
    outs = []
    for i in range(NCORES):
        o = res.results[i]["out"]  # [C, NODES] with node index = b*NLOC + n
        outs.append(o.reshape(C, B, NLOC).transpose(1, 0, 2))  # [B, C, NLOC]
    return np.concatenate(outs, axis=2)  # [B, C, N]


# revision 25
# speedup vs baseline: 1.3241x; 1.0992x over previous
"""GAT layer kernel for Trainium2, 8 NeuronCores, sharded over the node dim N.

reference computation (per (b, n) node, K=10 neighbors, H=16 heads, d=4):
    hw[k, h, d]  = sum_f x[b,n,k,f] * W[f, h*4+d]
    logit[k, h]  = sum_d hw[k,h,d] * a[d,h]
    attn         = softmax_k(logit)                  (max-subtraction skipped:
                                                      |logit| <~ 6 for randn inputs)
    out[h, d]    = leaky_relu(sum_k attn[k,h] * hw[k,h,d], 0.2)
    output layout [B, 4H, N]

Device strategy per core ("slab-10" layout, rows = B*N_loc*K = 250000):
  - node-blocks of 128 nodes (1280 rows); DMA loads a block with
    partition = node (10 rows = 5KB contiguous per partition) with
    fp32->bf16 cast (SWDGE)
  - PE transpose of [128 nodes, 128 f] per k -> xT block [f, (k, n)]
    (k-major), staged via PSUM + one ACT copy per block
  - per block-PAIR: matmuls pack the two blocks onto the 128 psum
    partitions via tile_position col-groups (A -> lanes 0-63,
    B -> 64-127) for both hw (lhsT=W) and logits (lhsT=Wa4, `a`
    folded into W on host, replicated x4 to align with hw channels)
  - ACT: p = exp(logits); DVE: q = p * hw; k-sum as a pairwise add
    tree (stage 1 in bf16 at the DVE 2x rate, k-major makes slices
    4B-aligned); reciprocal_approx + divide + leaky; out [64, nodes]
"""

import numpy as np

try:
    import concourse.bass as bass  # noqa: F401
except ImportError:  # toolchain lives in /opt on these boxes
    import sys

    for p in ("/opt/trn_rl_repo", "/root/.axon_site/_ro/trn_rl_repo"):
        sys.path.insert(0, p)
    import concourse.bass as bass

import concourse.bacc as bacc
import concourse.tile as tile
from concourse import mybir
from concourse.bass_utils import run_bass_kernel_spmd
from concourse.masks import make_identity

# problem shape (hardcoded per contest rules)
B, N, K, F, H = 4, 50000, 10, 128, 16
C = 4 * H  # 64 output channels
NCORES = 8
NLOC = N // NCORES  # 6250 nodes per core
ROWS = B * NLOC * K  # 250000 rows per core
NODES = B * NLOC  # 25000 nodes per core

NB = 128  # nodes per block
RB = NB * K  # rows per block (1280)
NFULL = NODES // NB  # 195 full blocks
NTAIL = NODES - NFULL * NB  # 40 nodes in the final partial block
NBLK = NFULL + 1  # 196 blocks
NPAIR = NBLK // 2  # 98 pairs
PAIRS_PER_CHUNK = 4

F32 = mybir.dt.float32
BF16 = mybir.dt.bfloat16
AX = mybir.AxisListType
OP = mybir.AluOpType
AF = mybir.ActivationFunctionType

# k-slices per block for the matmul/psum stage: (k0, nk) with nk*128 <= 512
KSLICES = [(0, 4), (4, 4), (8, 2)]


def build_kernel():
    nc = bacc.Bacc("TRN2", target_bir_lowering=False, debug=False)

    x_d = nc.dram_tensor("x", [ROWS, F], F32, kind="ExternalInput")
    wt_d = nc.dram_tensor("wt", [F, 2 * C], BF16, kind="ExternalInput")
    out_d = nc.dram_tensor("out", [C, NODES], F32, kind="ExternalOutput")

    from contextlib import ExitStack

    with tile.TileContext(nc) as tc, ExitStack() as ctx:
        consts = ctx.enter_context(tc.tile_pool(name="consts", bufs=1))
        xnat_p = ctx.enter_context(tc.tile_pool(name="xnat", bufs=2))
        xt_p = ctx.enter_context(tc.tile_pool(name="xt", bufs=2))
        tp_p = ctx.enter_context(tc.tile_pool(name="tp", bufs=2, space="PSUM"))
        hwl_p = ctx.enter_context(tc.tile_pool(name="hwl", bufs=2, space="PSUM"))
        lg_p = ctx.enter_context(tc.tile_pool(name="lg", bufs=2, space="PSUM"))
        p4_p = ctx.enter_context(tc.tile_pool(name="p4", bufs=2))
        q_p = ctx.enter_context(tc.tile_pool(name="q", bufs=2))
        t5_p = ctx.enter_context(tc.tile_pool(name="t5", bufs=1))
        ns_p = ctx.enter_context(tc.tile_pool(name="ns", bufs=2))
        out_p = ctx.enter_context(tc.tile_pool(name="outp", bufs=3))

        wt_sb = consts.tile([F, 2 * C], BF16)
        nc.sync.dma_start(out=wt_sb, in_=wt_d[:, :])
        ident = consts.tile([128, 128], BF16)
        make_identity(nc, ident)

        w_ap = wt_sb[:, 0:C]
        wa_ap = wt_sb[:, C : 2 * C]

        n_chunks = -(-NPAIR // PAIRS_PER_CHUNK)  # 25
        for ci in range(n_chunks):
            pr0 = ci * PAIRS_PER_CHUNK
            npr = min(PAIRS_PER_CHUNK, NPAIR - pr0)
            b0 = 2 * pr0  # first block of chunk
            nblk = 2 * npr
            nfull = min(nblk, NFULL - b0)  # full blocks in this chunk
            r0 = b0 * RB

            # ---- load x, partition = node, 5KB contiguous per partition ----
            xnat = xnat_p.tile([128, 2 * PAIRS_PER_CHUNK, K * F], BF16,
                               tag="xnat")
            if nfull:
                src = x_d[r0 : r0 + nfull * RB, :].rearrange(
                    "(blk p r) f -> p blk (r f)", p=NB, r=K
                )
                nc.gpsimd.dma_start(out=xnat[:, 0:nfull, :], in_=src)
            if nfull < nblk:  # partial final block (40 nodes)
                src = x_d[r0 + nfull * RB : ROWS, :].rearrange(
                    "(p r) f -> p (r f)", r=K
                )
                nc.gpsimd.dma_start(out=xnat[0:NTAIL, nfull, :], in_=src)

            # ---- transpose each block to k-major xT [f, (k, n)] ----
            xt = xt_p.tile([F, 2 * PAIRS_PER_CHUNK * K * NB], BF16, tag="xt")
            for blk in range(nblk):
                nn = NB if (b0 + blk) < NFULL else NTAIL
                tp = tp_p.tile([128, K * NB], BF16, tag="tp")
                for k in range(K):
                    nc.tensor.transpose(
                        tp[:, k * NB : k * NB + nn],
                        xnat[0:nn, blk, k * F : (k + 1) * F],
                        ident[0:nn, 0:nn],
                    )
                nc.scalar.copy(
                    out=xt[:, blk * K * NB : (blk + 1) * K * NB], in_=tp
                )

            # ---- per pair: matmuls + exp + attention multiply ----
            p4c = p4_p.tile([128, PAIRS_PER_CHUNK, K * NB], BF16, tag="p4")
            qc = q_p.tile([128, PAIRS_PER_CHUNK, K * NB], BF16, tag="q")
            for pj in range(npr):
                xA = xt[:, (2 * pj) * K * NB : (2 * pj + 1) * K * NB]
                xB = xt[:, (2 * pj + 1) * K * NB : (2 * pj + 2) * K * NB]
                for k0, nk in KSLICES:
                    s = k0 * NB
                    w = nk * NB
                    hwl = hwl_p.tile([128, 512], F32, tag="hwl")
                    lg = lg_p.tile([128, 512], F32, tag="lg")
                    nc.tensor.matmul(
                        hwl[0:C, 0:w], w_ap, xA[:, s : s + w],
                        tile_position=(0, 0),
                    )
                    nc.tensor.matmul(
                        hwl[C:128, 0:w], w_ap, xB[:, s : s + w],
                        tile_position=(0, 64),
                    )
                    nc.tensor.matmul(
                        lg[0:C, 0:w], wa_ap, xA[:, s : s + w],
                        tile_position=(0, 0),
                    )
                    nc.tensor.matmul(
                        lg[C:128, 0:w], wa_ap, xB[:, s : s + w],
                        tile_position=(0, 64),
                    )
                    nc.scalar.activation(
                        p4c[:, pj, s : s + w], lg[:, 0:w], AF.Exp
                    )
                    nc.vector.tensor_mul(
                        qc[:, pj, s : s + w], p4c[:, pj, s : s + w],
                        hwl[:, 0:w],
                    )

            # ---- k-sum via pairwise tree (stage 1 bf16 at DVE 2x) ----
            numer = ns_p.tile([128, PAIRS_PER_CHUNK, NB], F32, tag="numer")
            denom = ns_p.tile([128, PAIRS_PER_CHUNK, NB], F32, tag="denom")

            def ksum(src, dst, tag):
                v = src[:, 0:npr, :].rearrange("p j (k n) -> p j k n", k=K)
                t5 = t5_p.tile([128, PAIRS_PER_CHUNK, 5, NB], BF16,
                               tag=tag + "5")
                nc.vector.tensor_add(
                    t5[:, 0:npr], v[:, :, 0:5, :], v[:, :, 5:10, :]
                )
                t2 = t5_p.tile([128, PAIRS_PER_CHUNK, 2, NB], F32,
                               tag=tag + "2")
                nc.vector.tensor_add(
                    t2[:, 0:npr], t5[:, 0:npr, 0:2, :], t5[:, 0:npr, 2:4, :]
                )
                d = dst[:, 0:npr, :]
                nc.vector.tensor_add(
                    d, t2[:, 0:npr, 0, :], t2[:, 0:npr, 1, :]
                )
                nc.vector.tensor_add(d, d, t5[:, 0:npr, 4, :])

            ksum(qc, numer, "q")
            ksum(p4c, denom, "p")

            rinv = ns_p.tile([128, PAIRS_PER_CHUNK, NB], F32, tag="rinv")
            outc = out_p.tile([128, PAIRS_PER_CHUNK, NB], F32, tag="outc")
            nc.vector.reciprocal_approx_fast(
                rinv[:, 0:npr], denom[:, 0:npr]
            )
            nc.vector.tensor_mul(
                outc[:, 0:npr], numer[:, 0:npr], rinv[:, 0:npr]
            )
            nc.vector.scalar_tensor_tensor(
                out=outc[:, 0:npr], in0=outc[:, 0:npr], scalar=0.2,
                in1=outc[:, 0:npr], op0=OP.mult, op1=OP.max,
            )

            # ---- store: lanes 0-63 = even blocks, 64-127 = odd blocks ----
            n0 = b0 * NB
            if (b0 + nblk) <= NFULL:  # all blocks full: 2 batched DMAs
                dst = out_d[:, n0 : n0 + npr * 2 * NB].rearrange(
                    "c (j n2) -> c j n2", n2=2 * NB
                )
                nc.sync.dma_start(out=dst[:, :, 0:NB], in_=outc[0:C, 0:npr, :])
                nc.sync.dma_start(
                    out=dst[:, :, NB : 2 * NB], in_=outc[C:128, 0:npr, :]
                )
            else:  # final chunk contains the partial block: per-pair DMAs
                for pj in range(npr):
                    nA = n0 + pj * 2 * NB
                    nc.sync.dma_start(
                        out=out_d[:, nA : nA + NB], in_=outc[0:C, pj, :]
                    )
                    nB_ = min(NB, NODES - (nA + NB))
                    nc.sync.dma_start(
                        out=out_d[:, nA + NB : nA + NB + nB_],
                        in_=outc[C:128, pj, 0:nB_],
                    )

    nc.compile()
    return nc


_NC = None


def _get_nc():
    global _NC
    if _NC is None:
        _NC = build_kernel()
    return _NC


def kernel(x, W, a):
    import ml_dtypes

    x = np.asarray(x, dtype=np.float32)
    W = np.asarray(W, dtype=np.float32)
    a = np.asarray(a, dtype=np.float32)

    # fold `a` into W: Wa[f, h] = sum_d W[f, h*4+d] * a[d, h]; replicate x4
    Wr = W.reshape(F, H, 4)
    Wa = np.einsum("fhd,dh->fh", Wr.astype(np.float64), a.astype(np.float64))
    Wa4 = np.repeat(Wa, 4, axis=1)  # [F, 64], col h*4+d -> Wa[:, h]
    wt = np.concatenate([W, Wa4.astype(np.float32)], axis=1)
    wt_bf16 = wt.astype(ml_dtypes.bfloat16)

    nc = _get_nc()
    in_maps = []
    for i in range(NCORES):
        xi = np.ascontiguousarray(x[:, i * NLOC : (i + 1) * NLOC])
        in_maps.append({"x": xi.reshape(ROWS, F), "wt": wt_bf16})

    res = run_bass_kernel_spmd(nc, in_maps, core_ids=list(range(NCORES)))
    outs = []
    for i in range(NCORES):
        o = res.results[i]["out"]  # [C, NODES] with node index = b*NLOC + n
        outs.append(o.reshape(C, B, NLOC).transpose(1, 0, 2))  # [B, C, NLOC]
    return np.concatenate(outs, axis=2)  # [B, C, N]
